# revision 54
# baseline (speedup 1.0000x reference)
"""MRU encoding kernel for Trainium2 (8 NeuronCores, batch-parallel).

Problem (B=32, T=2048, D=300):
    z = tanh(x @ Wz.T + bz); o = tanh(x @ Wo.T + bo)
    c_t = g_t*c_{t-1} + (1-g_t)*z_t   (c_{-1}=0, scan over T)
    out = o * c

Per-core (4 batch rows) layout is [channel, time].

Matmul path — split-precision fp8 DoubleRow (2x PE rate, and each DR pass
covers 256 contraction rows):
    W' = e4m3(W*64), dW = e4m3(W*64 - W'), x' = e4m3(x*2), dx = e4m3(x*2 - x')
    psum = (W'+dW) @ x' + W'[:300] @ dx          (fp32 accumulate)
    z    = tanh(psum * 2^-7)                     (ACT scale undoes 64*2)
The *64 pre-scale keeps the W residual out of e4m3's subnormal dead zone
(without it the residual quantizes to +-2^-9 and the scheme loses 30x
accuracy).  The contraction stack is 646 rows = x'(301, incl. ones row
for the bias) + dx(300) + x'-tail dup(45); the dup rides in k-tile 5's
padding so dW's ragged tail pairs with W''s ragged tail in one DR pass:
    pass A: (t0,t1)=x'[0:256]   x  W'[0:256]
    pass B: (t0,t1)             x  dW[0:256]        (same moving tiles)
    pass C: (t2,t3)=[x'tail; dx[0:128]] x [W'tail; W''[0:128]]
    pass D: (t4,t5)=[dx[128:256]; dx tail | x' dup] x [W''[128:256]; W'' tail | dW tail]
4 DR passes/(row,proj,m-tile) at 0.5 cyc/col vs fp16's 3 passes at 1.0.

Elementwise path (per 128-channel chain): the chain spine gm1=g-1 (TS),
bneg=gm1*z (TT), scan (state=g*state+bneg gives -c) stays on DVE; the
DAG-leaf res=(-o)*(-c) runs on Pool for all but the last 2 chains
(gpsimd TensorTensor is in the standard Q7 library, and Pool/DVE run
concurrently).  o is produced NEGATED via tanh(scale=-2^-7).  Stores
are deferred ~3 chains (ACT ring) so they never block tanh dispatch;
late chains store via the then-idle SP ring immediately.

g rides fp16; the ragged gate lanes of each row pair are host-packed
into one [108,T] tile with 0.5-filled pad lanes (no device memsets).
x-pad lanes of k-tiles 2/5 are zeroed once per tile-pool buffer from a
small zeros input (stale NaNs would poison the PE even against zero
weights).

Rings: input loads ride the SP HWDGE ring; weights+stores ride the ACT
ring (HWDGE is FIFO per issuing engine; stores must not block prefetch).
"""

import numpy as np
import ml_dtypes

import concourse.bass as bass
import concourse.mybir as mybir
import concourse.tile as tile
from concourse import bacc
from concourse.bass_utils import run_bass_kernel_spmd

B, T, D = 32, 2048, 300
NCORES = 8
BC = B // NCORES  # 4 batch rows per core
DP = D + 1  # ones-row at index 300 carries the bias
DPAD = 320  # weight e-columns padded so the ragged chunk is m=64
TS = 512  # moving-operand max free dim
NT = T // TS
F32 = mybir.dt.float32
F16 = mybir.dt.float16
F8 = mybir.dt.float8e4
DR = mybir.MatmulPerfMode.DoubleRow

WS = 64.0  # weight pre-scale (keeps dW residual in e4m3 normal range)
XS = 2.0  # x pre-scale
ASCALE = 1.0 / (WS * XS)  # activation scale undoes both

XROWS = DP + D + 45  # 646: x'(301) + dx(300) + x'-tail dup(45)

# scheduling knobs (sim-tuned; the timeline scheduler is order-sensitive)
CFG = {
    "wsplit": False,   # split wz/wo head DMAs into (0:2, 2:8) pieces
    "xp_bufs": 3,
    "zp_bufs": 3,
    "ep_bufs": 3,
    "gp_bufs": 2,
    "ps_bufs": 2,
    "store_defer": 3,  # chains of lag before a store is issued on ACT ring
    "tail_tsplit": 2,
    "res_dve_tail": 2,  # how many final chains keep res on DVE
    "pads_late": False,  # zz pad DMAs after L1/L2
    "late_store_sp": True,  # chains >= late_store_from store via SP ring
    "late_store_from": 4,
    "g_late": True,  # load g after both rows' x loads
    "mm_tb_outer": False,  # bank-major matmul emission (banks close early)
    "all_tsplit": 1,  # T-split for every non-tail chain
    "wr_after": 2,  # emit wzr/wor after this many pair-0 chains
    "tail_store_now": True,  # tail-chain stores flush immediately (SP ring)
    "tanh_split": 1,  # T-split for tanh only
    "ps_half": False,  # [128,1024] psum tiles, half-granular handoff
}

_CACHE: dict = {}


def _build_program(reps=1, cfg=None):
    c = dict(CFG)
    if cfg:
        c.update(cfg)
    nc = bacc.Bacc("TRN2", target_bir_lowering=False, debug=False, num_devices=NCORES)

    d_x = nc.dram_tensor("xt", [BC, XROWS, T], F8, kind="ExternalInput").ap()
    d_g = nc.dram_tensor("gt", [BC, 256, T], F16, kind="ExternalInput").ap()
    d_g2 = nc.dram_tensor("g2", [BC // 2, 108, T], F16, kind="ExternalInput").ap()
    d_wz = nc.dram_tensor("wz", [128, 8, 256], F8, kind="ExternalInput").ap()
    d_wo = nc.dram_tensor("wo", [128, 8, 256], F8, kind="ExternalInput").ap()
    # ragged-column variants: DR matmuls may not target psum partition base
    # 64, so the pair's ragged outputs ride zero-padded full-128-col
    # stationaries (b0 in cols 0..63, b1 in cols 64..127) into one psum group
    d_wzr = nc.dram_tensor("wzr", [128, 8, 256], F8, kind="ExternalInput").ap()
    d_wor = nc.dram_tensor("wor", [128, 8, 256], F8, kind="ExternalInput").ap()
    d_zz = nc.dram_tensor("zz", [83, T], F8, kind="ExternalInput").ap()
    d_out0 = nc.dram_tensor("outt", [BC, D, T], F16, kind="ExternalOutput").ap()
    d_outs = [d_out0] * reps

    with tile.TileContext(nc) as tc:
        with (
            tc.tile_pool(name="wp", bufs=1) as wp,
            tc.tile_pool(name="xp", bufs=c["xp_bufs"]) as xp,
            tc.tile_pool(name="gp", bufs=c["gp_bufs"]) as gp,
            tc.tile_pool(name="zp", bufs=c["zp_bufs"]) as zp,
            tc.tile_pool(name="ep", bufs=c["ep_bufs"]) as ep,
            tc.tile_pool(name="ps", bufs=c["ps_bufs"], space="PSUM") as ps,
        ):
            # wz/wo ride the scalar ring first (the first matmuls need them);
            # the ragged variants are emitted after the first pair's loads so
            # they don't delay the head
            wts = {}

            def load_w(nm, dram, eng, split=False):
                t = wp.tile([128, 8, 256], F8, tag=nm, name=f"w_{nm}")
                if split:
                    # k-tiles 0:2 first: they're all the first (A) passes need
                    eng.dma_start(t[:, 0:2, :], dram[:, 0:2, :])
                    eng.dma_start(t[:, 2:8, :], dram[:, 2:8, :])
                else:
                    eng.dma_start(t[:, :, :], dram[:, :, :])
                wts[nm] = t

            load_w("wz", d_wz, nc.scalar, split=c["wsplit"])
            load_w("wo", d_wo, nc.scalar, split=c["wsplit"])

            xbuf_zeroed = [False] * c["xp_bufs"]

            def load_x(b, xbuf_idx):
                xt = xp.tile([128, 6, T], F8, tag="x", name="xt_t")

                def pads():
                    if xbuf_zeroed[xbuf_idx]:
                        return
                    # pad lanes of k-tiles 2/5 must be finite for the PE
                    # (zero-weight rows still multiply the moving data);
                    # zero them once per physical buffer via DMA
                    nc.sync.dma_start(xt[45:128, 2, :], d_zz[0:83, :])
                    nc.sync.dma_start(xt[44:64, 5, :], d_zz[0:20, :])
                    nc.sync.dma_start(xt[109:128, 5, :], d_zz[0:19, :])
                    xbuf_zeroed[xbuf_idx] = True

                if not c["pads_late"]:
                    pads()
                # x' k0/k1 in T-halves so the first matmuls start sooner
                nc.sync.dma_start(
                    xt[:, 0:2, 0:1024],
                    d_x[b, 0:256, 0:1024].rearrange("(c p) t -> p c t", c=2),
                )
                nc.sync.dma_start(
                    xt[:, 0:2, 1024:T],
                    d_x[b, 0:256, 1024:T].rearrange("(c p) t -> p c t", c=2),
                )
                if c["pads_late"]:
                    pads()
                nc.sync.dma_start(xt[0:45, 2, :], d_x[b, 256:301, :])
                nc.sync.dma_start(
                    xt[:, 3:5, :],
                    d_x[b, 301:557, :].rearrange("(c p) t -> p c t", c=2),
                )
                nc.sync.dma_start(xt[0:44, 5, :], d_x[b, 557:601, :])
                nc.sync.dma_start(xt[64:109, 5, :], d_x[b, 601:646, :])
                return xt

            KSLICES = (((0, 2), (0, 2)), ((2, 4), (0, 2)),
                       ((4, 6), (2, 4)), ((6, 8), (4, 6)))

            def proj_mms(psum, w, xt, msl, start=True, stop=True):
                """4 DoubleRow passes accumulating one projection into
                psum[0:128, :]. msl is the stationary column slice."""
                if c["mm_tb_outer"]:
                    loop = [(kk, tb) for tb in range(NT) for kk in range(4)]
                else:
                    loop = [(kk, tb) for kk in range(4) for tb in range(NT)]
                for kk, tb in loop:
                    wsl, xsl = KSLICES[kk]
                    nc.tensor.matmul(
                        psum[0:128, bass.ts(tb, TS)],
                        lhsT=w[:, wsl[0] : wsl[1], msl],
                        rhs=xt[:, xsl[0] : xsl[1], bass.ts(tb, TS)],
                        start=(start and kk == 0),
                        stop=(stop and kk == 3),
                        perf_mode=DR,
                    )

            def proj_mms_h(halves, w, xt, msl, hf, start=True, stop=True):
                """Half-psum variant: fill half-tile hf (T cols
                hf*1024..hf*1024+1024) so each half closes at 50% of a
                fill and frees psum earlier."""
                for kk in range(4):
                    wsl, xsl = KSLICES[kk]
                    for tb in (2 * hf, 2 * hf + 1):
                        nc.tensor.matmul(
                            halves[hf][0:128, bass.ts(tb - 2 * hf, TS)],
                            lhsT=w[:, wsl[0] : wsl[1], msl],
                            rhs=xt[:, xsl[0] : xsl[1], bass.ts(tb, TS)],
                            start=(start and kk == 0),
                            stop=(stop and kk == 3),
                            perf_mode=DR,
                        )

            # stores are deferred a few chains so the ACT sequencer never
            # blocks on a not-yet-produced res while tanh work queues behind
            # it; late chains' stores ride the SP ring (idle once loads done)
            pending_stores = []  # (chain, pool_res, ds, res_tile, lo, hi, hs)
            chain_no = [0]

            def flush_stores(upto, final=False):
                due = [e for e in pending_stores if e[0] <= upto]
                if final:
                    # within the tail, Pool-produced res lands latest: put
                    # those stores behind the DVE-produced ones on the ring
                    due.sort(key=lambda e: (e[1], e[0]))
                for e in due:
                    pending_stores.remove(e)
                    ch, _, ds, rt, lo, hi, hs = e
                    ring = (nc.sync if (c["late_store_sp"] and ch >= c["late_store_from"])
                            else nc.scalar)
                    ring.dma_start(ds[:, hs], rt[lo:hi, hs])

            def elemwise(gs, z_ap, oneg_ap, mj, stores, tsplit=1):
                """bneg=(g-1)z -> scan(-c) -> out = (-o)*(-c); stores is a
                list of (res_slice, dram_slice).  The chain spine
                (gm1/bneg/scan) stays on DVE; res is a DAG leaf and runs on
                Pool (Pool and DVE overlap in the model) except for the last
                res_dve_tail chains.  tsplit>1 pipelines the chain in
                T-chunks for the kernel tail."""
                cur = chain_no[0]
                chain_no[0] += 1
                res_pool = (cur % 10) < 10 - c["res_dve_tail"]
                flush_stores(cur - c["store_defer"])
                gm1 = ep.tile([128, T], F16, tag="gm1", name="gm1_t")
                bneg = ep.tile([128, T], F16, tag="bneg", name="bneg_t")
                cneg = ep.tile([128, T], F16, tag="c", name="cneg_t")
                res = ep.tile([128, T], F16, tag="res", name="res_t")
                tw = T // tsplit
                for h in range(tsplit):
                    hs = slice(h * tw, (h + 1) * tw)
                    nc.vector.tensor_scalar_add(gm1[:mj, hs], gs[:mj, hs], -1.0)
                    nc.vector.tensor_mul(bneg[:mj, hs], gm1[:mj, hs], z_ap[:mj, hs])
                    init = 0.0 if h == 0 else cneg[:mj, h * tw - 1 : h * tw]
                    nc.vector.tensor_tensor_scan(
                        cneg[:mj, hs], gs[:mj, hs], bneg[:mj, hs], init,
                        op0=mybir.AluOpType.mult, op1=mybir.AluOpType.add,
                    )
                    eng = nc.gpsimd if res_pool else nc.vector
                    eng.tensor_mul(res[:mj, hs], oneg_ap[:mj, hs], cneg[:mj, hs])
                    for rs, ds in stores:
                        if (c["tail_store_now"] and c["late_store_sp"]
                                and cur >= c["late_store_from"]):
                            nc.sync.dma_start(ds[:, hs], res[rs[0] : rs[1], hs])
                        else:
                            pending_stores.append(
                                (cur, res_pool, ds, res, rs[0], rs[1], hs)
                            )

            for d_out in d_outs:
              for pair in range(BC // 2):
                b0, b1 = 2 * pair, 2 * pair + 1
                xts = {}
                gts = {}
                for b in (b0, b1):
                    xts[b] = load_x(b, b % c["xp_bufs"])
                    if not c["g_late"]:
                        gt = gp.tile([128, 2, T], F16, tag="g", name="gt_t")
                        nc.sync.dma_start(
                            gt[:, :, :],
                            d_g[b, :, :].rearrange("(c p) t -> p c t", c=2),
                        )
                        gts[b] = gt
                if c["g_late"]:
                    for b in (b0, b1):
                        gt = gp.tile([128, 2, T], F16, tag="g", name="gt_t")
                        nc.sync.dma_start(
                            gt[:, :, :],
                            d_g[b, :, :].rearrange("(c p) t -> p c t", c=2),
                        )
                        gts[b] = gt
                # ragged gate lanes of the pair, host-packed (pads = 0.5)
                g2 = gp.tile([128, T], F16, tag="g2", name="g2_t")
                nc.sync.dma_start(g2[0:108, :], d_g2[pair, :, :])
                if pair == 0 and c["wr_after"] == 0:
                    load_w("wzr", d_wzr, nc.scalar)
                    load_w("wor", d_wor, nc.scalar)

                def do_j(b, j, tsplit=1):
                    m0 = 128 * j
                    msl = slice(m0, m0 + 128)
                    z_j = zp.tile([128, T], F16, tag="z", name="t_z")
                    oneg_j = zp.tile([128, T], F16, tag="o", name="t_o")
                    if c["ps_half"]:
                        pzh = [ps.tile([128, 1024], F32, tag="p", name="psz")
                               for _ in range(2)]
                        poh = [ps.tile([128, 1024], F32, tag="p", name="pso")
                               for _ in range(2)]
                        for hf in range(2):
                            proj_mms_h(pzh, wts["wz"], xts[b], msl, hf)
                        for hf in range(2):
                            proj_mms_h(poh, wts["wo"], xts[b], msl, hf)
                        for hf in range(2):
                            hs = slice(hf * 1024, hf * 1024 + 1024)
                            nc.scalar.activation(
                                z_j[:, hs], pzh[hf][:, :],
                                mybir.ActivationFunctionType.Tanh, scale=ASCALE,
                            )
                        for hf in range(2):
                            hs = slice(hf * 1024, hf * 1024 + 1024)
                            nc.scalar.activation(
                                oneg_j[:, hs], poh[hf][:, :],
                                mybir.ActivationFunctionType.Tanh, scale=-ASCALE,
                            )
                    else:
                        pz = ps.tile([128, T], F32, tag="p", name="psum_z")
                        po = ps.tile([128, T], F32, tag="p", name="psum_o")
                        proj_mms(pz, wts["wz"], xts[b], msl)
                        proj_mms(po, wts["wo"], xts[b], msl)
                        tsp = max(tsplit, c["tanh_split"])
                        tw2 = T // tsp
                        for h in range(tsp):
                            hs = slice(h * tw2, (h + 1) * tw2)
                            nc.scalar.activation(
                                z_j[:, hs], pz[:, hs],
                                mybir.ActivationFunctionType.Tanh, scale=ASCALE,
                            )
                            nc.scalar.activation(
                                oneg_j[:, hs], po[:, hs],
                                mybir.ActivationFunctionType.Tanh, scale=-ASCALE,
                            )
                    elemwise(
                        gts[b][:, j, :], z_j[:, :], oneg_j[:, :], 128,
                        [((0, 128), d_out[b, m0 : m0 + 128, :])],
                        tsplit=tsplit,
                    )

                do_j(b0, 0, tsplit=c["all_tsplit"])
                if pair == 0 and c["wr_after"] == 1:
                    load_w("wzr", d_wzr, nc.scalar)
                    load_w("wor", d_wor, nc.scalar)
                do_j(b0, 1, tsplit=c["all_tsplit"])
                if pair == 0 and c["wr_after"] == 2:
                    load_w("wzr", d_wzr, nc.scalar)
                    load_w("wor", d_wor, nc.scalar)

                # ragged e-rows 256..299 of BOTH batch rows share one tile:
                # b0 at psum partitions 0..63 (wr cols 0:128), b1 at 64..127
                # (wr cols 128:256); the zero column halves make each row's
                # passes add 0 to the other's region within one psum group
                z2 = zp.tile([128, T], F16, tag="z", name="t_z2")
                oneg2 = zp.tile([128, T], F16, tag="o", name="t_o2")
                if c["ps_half"]:
                    pz2h = [ps.tile([128, 1024], F32, tag="p", name="psz2")
                            for _ in range(2)]
                    po2h = [ps.tile([128, 1024], F32, tag="p", name="pso2")
                            for _ in range(2)]
                    for ph, w_ in ((pz2h, "wzr"), (po2h, "wor")):
                        for hf in range(2):
                            proj_mms_h(ph, wts[w_], xts[b0], slice(0, 128),
                                       hf, stop=False)
                            proj_mms_h(ph, wts[w_], xts[b1], slice(128, 256),
                                       hf, start=False)
                    for hf in range(2):
                        hs = slice(hf * 1024, hf * 1024 + 1024)
                        nc.scalar.activation(
                            z2[:, hs], pz2h[hf][:, :],
                            mybir.ActivationFunctionType.Tanh, scale=ASCALE,
                        )
                    for hf in range(2):
                        hs = slice(hf * 1024, hf * 1024 + 1024)
                        nc.scalar.activation(
                            oneg2[:, hs], po2h[hf][:, :],
                            mybir.ActivationFunctionType.Tanh, scale=-ASCALE,
                        )
                else:
                    pz2 = ps.tile([128, T], F32, tag="p", name="psum_z2")
                    po2 = ps.tile([128, T], F32, tag="p", name="psum_o2")
                    proj_mms(pz2, wts["wzr"], xts[b0], slice(0, 128), stop=False)
                    proj_mms(pz2, wts["wzr"], xts[b1], slice(128, 256), start=False)
                    proj_mms(po2, wts["wor"], xts[b0], slice(0, 128), stop=False)
                    proj_mms(po2, wts["wor"], xts[b1], slice(128, 256), start=False)
                    nc.scalar.activation(
                        z2[:, :], pz2[:, :],
                        mybir.ActivationFunctionType.Tanh, scale=ASCALE,
                    )
                    nc.scalar.activation(
                        oneg2[:, :], po2[:, :],
                        mybir.ActivationFunctionType.Tanh, scale=-ASCALE,
                    )
                elemwise(
                    g2[:, :], z2[:, :], oneg2[:, :], 108,
                    [((0, 44), d_out[b0, 256:D, :]),
                     ((64, 108), d_out[b1, 256:D, :])],
                )

                last = pair == BC // 2 - 1
                do_j(b1, 0, tsplit=c["all_tsplit"])
                # the kernel's very last chain is split in T-chunks so its
                # scan/mul/store pipeline instead of dangling serially
                do_j(b1, 1, tsplit=c["tail_tsplit"] if last else 1)

              flush_stores(10**9, final=True)

    nc.compile()
    return nc


def _quant8(a):
    return a.astype(ml_dtypes.float8_e4m3)


def make_in_maps(gate_encoding, inputs_encoding, Wz, bz, Wo, bo):
    """Host-side prep (untimed): fp8 split-quantization + tile packing."""
    e4 = ml_dtypes.float8_e4m3

    def w_img(Wm, bv):
        Wa = np.zeros((DP, DPAD), dtype=np.float32)
        Wa[:D, :D] = Wm.T
        Wa[D, :D] = bv
        Wa *= WS
        W8 = _quant8(Wa).astype(np.float32)
        dW8 = _quant8(Wa - W8).astype(np.float32)
        img = np.zeros((128, 8, DPAD), dtype=np.float32)
        img[:, 0, :] = W8[0:128]
        img[:, 1, :] = W8[128:256]
        img[:, 2, :] = dW8[0:128]
        img[:, 3, :] = dW8[128:256]
        img[0:45, 4, :] = W8[256:301]          # C0: W' tail
        img[:, 5, :] = W8[0:128]               # C1: W'' head (bias row is
        img[:, 6, :] = W8[128:256]             # D0: W'' mid    beyond 300 so
        img[0:44, 7, :] = W8[256:300]          # D1 lo: W'' tail  W''==W'[:300])
        img[64:109, 7, :] = dW8[256:301]       # D1 hi: dW tail (x' dup lanes)
        base = img[:, :, 0:256].astype(e4)
        # ragged variant: the stack's e-columns 256..299 land in stationary
        # cols 0..43 (b0 block) and 64..107 (b1 block); other cols zero
        rag = np.zeros((128, 8, 256), dtype=np.float32)
        rag[:, :, 0:44] = img[:, :, 256:300]
        rag[:, :, 128 + 64 : 128 + 108] = img[:, :, 256:300]
        return base, rag.astype(e4)

    wz_img, wzr_img = w_img(Wz, bz)
    wo_img, wor_img = w_img(Wo, bo)

    in_maps = []
    for cc in range(NCORES):
        xs = inputs_encoding[cc * BC : (cc + 1) * BC]  # [BC, T, D]
        gs = gate_encoding[cc * BC : (cc + 1) * BC]
        xt = np.empty((BC, XROWS, T), dtype=e4)
        gt = np.empty((BC, 256, T), dtype=np.float16)
        g2 = np.full((BC // 2, 108, T), 0.5, dtype=np.float16)
        for b in range(BC):
            xa = np.empty((DP, T), dtype=np.float32)
            xa[:D] = xs[b].T * XS
            xa[D] = XS
            x8 = _quant8(xa)
            dx8 = _quant8(xa - x8.astype(np.float32))
            xt[b, 0:DP] = x8
            xt[b, DP : DP + D] = dx8[0:D]
            xt[b, DP + D :] = x8[256:DP]
            gT = gs[b].T.astype(np.float16)  # [D, T]
            gt[b] = gT[0:256]
            g2[b // 2, (b % 2) * 64 : (b % 2) * 64 + 44] = gT[256:D]
        in_maps.append({
            "xt": xt, "gt": gt, "g2": g2,
            "wz": wz_img, "wo": wo_img, "wzr": wzr_img, "wor": wor_img,
            "zz": np.zeros((83, T), dtype=e4),
        })
    return in_maps


def kernel(gate_encoding, inputs_encoding, Wz, bz, Wo, bo):
    gate_encoding = np.asarray(gate_encoding, dtype=np.float32)
    inputs_encoding = np.asarray(inputs_encoding, dtype=np.float32)
    Wz = np.asarray(Wz, dtype=np.float32)
    bz = np.asarray(bz, dtype=np.float32)
    Wo = np.asarray(Wo, dtype=np.float32)
    bo = np.asarray(bo, dtype=np.float32)

    if "nc" not in _CACHE:
        _CACHE["nc"] = _build_program()
    nc = _CACHE["nc"]

    in_maps = make_in_maps(gate_encoding, inputs_encoding, Wz, bz, Wo, bo)
    res = run_bass_kernel_spmd(nc, in_maps, core_ids=list(range(NCORES)))

    out = np.empty((B, T, D), dtype=np.float32)
    for cc in range(NCORES):
        out[cc * BC : (cc + 1) * BC] = (
            res.results[cc]["outt"].transpose(0, 2, 1).astype(np.float32)
        )
    return out
